# revision 22
# baseline (speedup 1.0000x reference)
"""CodeGen attention (B=2, S=2048, E=4096, H=16, HD=256) on 8 trn2 NeuronCores.

Sharding: data parallel over batch (2) x tensor parallel over heads (4 groups
of 4 heads) = 8 cores. Each core projects its 4 heads' q/k/v (W columns
pre-gathered on host), applies rotary embedding, and runs causal attention.

v3 (PE diet): the kernel is tensor-engine bound (98% busy in v2), so v3
removes PE work and trims the cold edges:
  - softmax denominator: instead of a ones-matmul per (head, k-tile) unit
    (160 N=512 streams), exp tiles accumulate on the idle DVE into an SBUF
    running sum; ONE ones-matmul per query block (16 total) reduces the
    128-partition sum. The den matmul + normalize for block B are deferred
    to block B+1's start so the PE never waits on the DVE accumulator.
  - V naturalization: PE transposes (128 matmuls + 128 ScalarE PSUM
    evictions) replaced by 2 DMA xbar transposes per head (fp16-width
    path, SBUF->SBUF, issued one head ahead on the ACT HWDGE queue).
  - exp weights in bf16 (was f32r): same exponent range as f32 so the
    logit tails can't overflow (fp16 would), half the DVE mask cost and
    the AV matmuls stream 2-byte operands. V is bf16 for the same reason
    (dtype-matched AV matmul). Softmax-weight mantissa (8 bits) costs
    ~2e-3 rel_rms on the output, well inside the 2e-2 gate.
  - startup: W f-tile 0 + X chunk DMAs issue before the mask/table
    constants so the first matmul isn't queued behind 1 MB of masks.

Device dataflow per core:
  projection: per token half, X^T fp16 chunks (32 x [128, 1024]) resident
    (ACT HWDGE queue so WAR-gated reloads don't block W prefetches on the
    sync queue); stream W fp16 f-tiles; accumulate PSUM (128 f x 1024 s)
    over 32 contraction chunks, two N=512 matmuls per chunk (PSUM bank
    limit); RoPE fused into the PSUM->SBUF eviction (features
    pre-deinterleaved per head so rotation is elementwise against sin/cos
    tables); K f-tiles (fp16) and V f-tiles (bf16) write straight into
    resident SBUF buffers, Q f-tiles spill to DRAM fp16. Pairs emitted
    grouped by head (q, k, v per head) so attention dependencies complete
    early.
  attention: per head, V DMA-transposed to natural (k, d) bf16 tiles one
    head ahead; per query block accumulate over causal k-tiles: P^T = K Q^T
    in PSUM -> exp (ScalarE, 1/16 scale and -4 shift folded, bf16 out) ->
    masked on diagonal tiles (DVE) -> DVE running sum for the denominator
    + A V accumulation (two N=512 matmuls per k-tile). AV trails the exp
    pipeline by two k-tiles so ScalarE/DVE latency stays hidden; each
    concurrently-pending PSUM accumulation group gets its own bank.
    Deferred per block: ones-matmul denominator reduce, DVE reciprocal,
    GpSimd normalize, DMA out (d x q).

Host reassembles the full (2, 2048, 4096) output from the per-core
(1024 features x 2048 tokens) transposed shards.
"""

import ml_dtypes
import numpy as np

import concourse.bass as bass
import concourse.tile as tile
from concourse import bacc, mybir
from concourse.bass_utils import run_bass_kernel_spmd

F32 = mybir.dt.float32
F32R = mybir.dt.float32r
F16 = mybir.dt.float16
BF16 = mybir.dt.bfloat16
MULT = mybir.AluOpType.mult
ADD = mybir.AluOpType.add
SUB = mybir.AluOpType.subtract

P = 128
B, S, E, H, HD = 2, 2048, 4096, 16, 256
NHC = 4            # heads per core
KK = E // P        # 32 contraction chunks
FT = 3 * NHC * HD // P   # 24 feature tiles per core (8 q, 8 k, 8 v)
SH = S // 2        # tokens per projection half
QB = 512           # query block in attention (N=512 matmuls hide LDWEIGHTS)
NQB = S // QB
KTQ = QB // P      # k-tiles (128 wide) per query block


def build_nc():
    nc = bacc.Bacc(None, target_bir_lowering=False, debug=False)

    hsT = nc.declare_dram_parameter("hsT", [KK, P, S], F16, isOutput=False)
    wt = nc.declare_dram_parameter("wt", [FT, P, KK, P], F16, isOutput=False)
    sinT = nc.declare_dram_parameter("sinT", [P, S], F32, isOutput=False)
    cosT = nc.declare_dram_parameter("cosT", [P, S], F32, isOutput=False)
    masks = nc.declare_dram_parameter("masks", [KTQ, P, QB], BF16, isOutput=False)
    out = nc.declare_dram_parameter("out", [2 * NHC, P, S], F32, isOutput=True)
    qT = nc.dram_tensor("qT", [2 * NHC, P, S], F16)

    # pair emission order: (q, v, k) per head so attention deps finish early
    # AND each head's V pair lands one pair before its K pair: the V xbar
    # transpose (issued right after the V RoPE) then drains off the sync
    # queue while the K pair's matmuls still run, instead of stalling the
    # attention-critical DMAs behind the transpose scratch serialization
    pairs = []
    for j in range(NHC):
        pairs += [(2 * j, 2 * j + 1), (16 + 2 * j, 16 + 2 * j + 1),
                  (8 + 2 * j, 8 + 2 * j + 1)]
    f_seq = [f for pr in pairs for f in pr]

    with tile.TileContext(nc) as tc:
        with (
            tc.tile_pool(name="kv", bufs=1) as kv_pool,
            tc.tile_pool(name="sm", bufs=1) as sm_pool,
            tc.tile_pool(name="vnp", bufs=1) as vn_pool,
            # attention's P^T PSUM pool opens before the projection pool so
            # it owns banks the projection never touches: the first QK
            # matmul must not wait on a PSUM WAR against the last RoPE
            tc.tile_pool(name="atp", bufs=2, space="PSUM") as at_psum,
        ):
            # resident K (0-7, fp16) and V (8-15, bf16) f-tiles
            # [128 feat x 2048 tok]
            kvt = [
                kv_pool.tile([P, S], F16 if i < 8 else BF16,
                             tag=f"kv{i}", name=f"kv{i}")
                for i in range(16)
            ]

            # attention constants (memsets are DVE, cheap, no DMA)
            ones_f = sm_pool.tile([P, P], F32)
            nc.vector.memset(ones_f[:], 1.0)
            ones = sm_pool.tile([P, P], F32R)
            nc.vector.tensor_copy(out=ones[:], in_=ones_f[:])
            nbias = sm_pool.tile([P, 1], F32)
            nc.vector.memset(nbias[:], -4.0)
            mask_t = sm_pool.tile([P, KTQ, QB], BF16)
            # head-0/block-0 Q kept resident (copied out of the RoPE staging
            # tiles) so the first attention matmul needs no qT roundtrip
            qt0 = sm_pool.tile([P, 2, QB], F16)

            # natural-layout V tiles, filled by DMA xbar transposes issued
            # from inside the projection loop as each head's V completes
            vns = {}

            def issue_vn(j):
                # vn{dc}[p, k2, d] = kvt_v[d, k2*128 + p]  (2-byte xbar path)
                # unique buffers per head: no WAR can delay a transpose into
                # the attention phase. Sync queue: ACT carries the exps.
                vn0 = vn_pool.tile([P, S // P, P], BF16, tag=f"vn0_{j}",
                                   name=f"vn0_{j}")
                vn1 = vn_pool.tile([P, S // P, P], BF16, tag=f"vn1_{j}",
                                   name=f"vn1_{j}")
                nc.sync.dma_start(
                    out=vn0[:], in_=kvt[8 + 2 * j][:], transpose=True
                )
                nc.sync.dma_start(
                    out=vn1[:], in_=kvt[8 + 2 * j + 1][:], transpose=True
                )
                vns[j] = (vn0, vn1)

            # PE warmup: the first real matmul waits ~11us for the X/W DMAs;
            # burn that idle on dummy matmuls so the HAM clock gate is at
            # 8/8 (2.4 GHz) when real work arrives (scratch PSUM bank from
            # the attention pt pool, which is unused during projection)
            warm = at_psum.tile([P, QB], F32, tag="pt", padded_shape=[None, 512])
            for _ in range(12):
                nc.tensor.matmul(warm[:, :P], ones[:], ones[:],
                                 start=True, stop=True)

            # ---------------- projection + RoPE ----------------
            # tokens processed in four 512-wide quarters with double-buffered
            # X group tiles: quarter t+1's X streams in while quarter t
            # computes, so only the first 4 MB of X gates the pipeline and
            # the quarter transitions cost nothing
            with (
                tc.tile_pool(name="xt", bufs=2) as xt_pool,
                tc.tile_pool(name="wst", bufs=2) as w_pool,
                tc.tile_pool(name="tab", bufs=1) as tab_pool,
                tc.tile_pool(name="rop", bufs=1) as rop_pool,
                tc.tile_pool(name="qst", bufs=1) as q_pool,
                tc.tile_pool(name="pjp", bufs=4, space="PSUM") as pj_psum,
            ):
                XG = 4           # X chunks per DMA group (one issue + sem per
                NXG = KK // XG   # group: the 0.6us/issue sequencer cost would
                                 # otherwise outrun the first f-tile's stream)
                SQ = QB          # 512 tokens per quarter

                def load_x(tq):
                    xs = []
                    for g in range(NXG):
                        t = xt_pool.tile([P, XG, SQ], F16, tag=f"xg{g}",
                                         name=f"xg{g}_{tq}")
                        # groups alternate across both HWDGE queues so the
                        # first f-tile's accumulation never outruns X
                        eng = nc.sync if g % 2 == 1 else nc.scalar
                        eng.dma_start(
                            out=t[:],
                            in_=hsT[XG * g:XG * (g + 1), :,
                                    tq * SQ:(tq + 1) * SQ]
                            .rearrange("k p t -> p k t"),
                        )
                        xs.append(t)
                    return xs

                xt_next = None
                for tq in range(S // SQ):
                    s0 = tq * SQ

                    def w_load(f, split=1, eng=None):
                        # split>1 halves the first-matmul wait at kernel start
                        w = w_pool.tile([P, KK, P], F16, tag="w")
                        step = KK // split
                        for q4 in range(split):
                            (eng or nc.sync).dma_start(
                                out=w[:, step * q4:step * (q4 + 1), :],
                                in_=wt[f, :, step * q4:step * (q4 + 1), :],
                            )
                        return w

                    w_q = [w_load(f_seq[0], split=2 if tq == 0 else 1)]
                    xt_g = load_x(0) if tq == 0 else xt_next
                    # f1 prefetch rides the ACT queue at startup so it never
                    # queues behind the odd X groups on sync
                    w_q.append(w_load(f_seq[1],
                                      eng=nc.scalar if tq == 0 else nc.sync))
                    if tq == 1:
                        # masks only needed at attention time: keep their
                        # 0.5 MB out of the startup DMA bandwidth
                        for ktl in range(KTQ):
                            nc.scalar.dma_start(
                                out=mask_t[:, ktl, :], in_=masks[ktl]
                            )

                    cs = sn = None
                    for pi, (fe, fo) in enumerate(pairs):
                        ps_e = pj_psum.tile([P, SQ], F32, tag="pj",
                                            padded_shape=[None, 512])
                        ps_o = pj_psum.tile([P, SQ], F32, tag="pj",
                                            padded_shape=[None, 512])
                        for f, ps in ((fe, ps_e), (fo, ps_o)):
                            w = w_q.pop(0)
                            fi = f_seq.index(f)
                            if fi >= 1 and fi + 1 < FT:
                                w_q.append(w_load(f_seq[fi + 1]))
                            for k in range(KK):
                                nc.tensor.matmul(
                                    ps[:], w[:, k, :],
                                    xt_g[k // XG][:, k % XG, :],
                                    start=(k == 0), stop=(k == KK - 1),
                                )
                        if pi == 0:
                            # tables emitted after the first W prefetches so
                            # their DMA never queues ahead of W at startup
                            # (first RoPE read is ~25us in, plenty of slack)
                            cs = tab_pool.tile([P, SQ], F32, tag="cs")
                            sn = tab_pool.tile([P, SQ], F32, tag="sn")
                            tab_eng = nc.scalar if tq == 0 else nc.sync
                            tab_eng.dma_start(out=cs[:], in_=cosT[:, s0:s0 + SQ])
                            tab_eng.dma_start(out=sn[:], in_=sinT[:, s0:s0 + SQ])
                        if pi == 5 and tq + 1 < S // SQ:
                            # next quarter's X prefetch: emitted mid-quarter
                            # (a head-of-queue WAR wait would block the queue,
                            # and at tq=0 this keeps it off the startup DMA
                            # bandwidth crunch) and double-buffered, so the
                            # data streams in while this quarter computes
                            xt_next = load_x(tq + 1)
                        # RoPE: oe = pe*cos - po*sin ; oo = po*cos + pe*sin
                        # (oe/oo double as temps; final DVE write rounds 16b)
                        t1 = rop_pool.tile([P, SQ], F32, tag="t1")
                        t3 = rop_pool.tile([P, SQ], F32, tag="t3")
                        if fe < 8:  # q pair: staging then spill to DRAM
                            oe = q_pool.tile([P, SQ], F16, tag="qe")
                            oo = q_pool.tile([P, SQ], F16, tag="qo")
                            oe_ap, oo_ap = oe[:], oo[:]
                        else:       # k/v pair: write into resident buffers
                            oe_ap = kvt[fe - 8][:, s0:s0 + SQ]
                            oo_ap = kvt[fo - 8][:, s0:s0 + SQ]
                        nc.vector.tensor_tensor(t1[:], ps_e[:], cs[:], MULT)
                        nc.vector.tensor_tensor(oo_ap, ps_e[:], sn[:], MULT)
                        nc.vector.tensor_tensor(oe_ap, ps_o[:], sn[:], MULT)
                        nc.vector.tensor_tensor(t3[:], ps_o[:], cs[:], MULT)
                        nc.vector.tensor_tensor(oe_ap, t1[:], oe_ap, SUB)
                        nc.vector.tensor_tensor(oo_ap, t3[:], oo_ap, ADD)
                        if fe < 8:
                            nc.sync.dma_start(out=qT[fe, :, s0:s0 + SQ], in_=oe[:])
                            nc.sync.dma_start(out=qT[fo, :, s0:s0 + SQ], in_=oo[:])
                            if tq == 0 and fe == 0:
                                # stash head-0 q (first QB tokens) in SBUF
                                nc.scalar.copy(out=qt0[:, 0, :], in_=oe[:])
                                nc.scalar.copy(out=qt0[:, 1, :], in_=oo[:])
                        elif tq == 3 and fe >= 16:
                            # this head's V is complete: transpose it now so
                            # the xbar scratch serialization hides under the
                            # projection instead of stalling attention
                            issue_vn((fe - 16) // 2)

            # ---------------- attention ----------------
            with (
                tc.tile_pool(name="qtp", bufs=4) as qt_pool,
                tc.tile_pool(name="epp", bufs=5) as ep_pool,
                tc.tile_pool(name="esp", bufs=2) as es_pool,
                tc.tile_pool(name="onp", bufs=3) as on_pool,
                tc.tile_pool(name="avp", bufs=2, space="PSUM") as av_psum,
                tc.tile_pool(name="av1p", bufs=2, space="PSUM") as av1_psum,
                tc.tile_pool(name="dnp", bufs=2, space="PSUM") as dn_psum,
            ):
                # den matmul + normalize for block (j,qb) are emitted at the
                # next block's start so PE/DVE never gate each other.
                pending = []

                def flush_prev():
                    while pending:
                        pj, pq0, pav0, pav1, pes = pending.pop(0)
                        dn = dn_psum.tile(
                            [P, QB], F32, tag="dn", padded_shape=[None, 512]
                        )
                        nc.tensor.matmul(
                            dn[:], ones[:], pes[:], start=True, stop=True
                        )
                        rb = on_pool.tile([P, QB], F32, tag="rb")
                        # ~18 correct bits; softmax denominators are benign
                        nc.vector.reciprocal_approx_fast(rb[:], dn[:])
                        for dc, pav in ((0, pav0), (1, pav1)):
                            o = on_pool.tile([P, QB], F32, tag="o")
                            nc.vector.tensor_tensor(o[:], pav[:], rb[:], MULT)
                            nc.sync.dma_start(
                                out=out[2 * pj + dc, :, pq0:pq0 + QB], in_=o[:]
                            )

                # qt tiles prefetched one block ahead; block (0,0) reads the
                # SBUF-resident qt0 stash (no qT roundtrip)
                blocks = [(j, qb) for j in range(NHC) for qb in range(NQB)]

                def load_qt(j, qb):
                    qt = qt_pool.tile([P, 2, QB], F16, tag="qt")
                    for dc in range(2):
                        nc.sync.dma_start(
                            out=qt[:, dc, :],
                            in_=qT[2 * j + dc, :, qb * QB:(qb + 1) * QB],
                        )
                    return qt

                qt_next = None
                for bi, (j, qb) in enumerate(blocks):
                    if qb == 0:
                        vn0, vn1 = vns[j]
                        k0t, k1t = kvt[2 * j], kvt[2 * j + 1]
                    if True:
                        flush_prev()
                        q0 = qb * QB
                        qt = qt0 if bi == 0 else qt_next
                        if bi + 1 < len(blocks):
                            qt_next = load_qt(*blocks[bi + 1])
                        # one full PSUM bank per concurrently-pending
                        # accumulation group (group tracking is per bank)
                        av0 = av_psum.tile(
                            [P, QB], F32, tag="av0", padded_shape=[None, 512]
                        )
                        av1 = av1_psum.tile(
                            [P, QB], F32, tag="av1", padded_shape=[None, 512]
                        )
                        es = es_pool.tile([P, QB], F32R, tag="es")
                        nkt = KTQ * (qb + 1)

                        def emit_av(kti, ep, c0, st, sp):
                            nc.tensor.matmul(
                                av0[:, c0:], vn0[:, kti, :], ep[:, c0:],
                                start=st, stop=sp,
                            )
                            nc.tensor.matmul(
                                av1[:, c0:], vn1[:, kti, :], ep[:, c0:],
                                start=st, stop=sp,
                            )

                        # AV runs two k-tiles behind P^T/exp so the
                        # pt->exp->mask chain plus semaphore latency never
                        # gates the PE. Diagonal k-tile t only serves
                        # queries >= 128t: QK/exp/AV run at width 512-128t
                        # (causal triangle trimmed at k-tile granularity).
                        pend = []
                        for kti in range(nkt):
                            dt = kti - KTQ * qb
                            c0 = dt * P if dt > 0 else 0
                            pt = at_psum.tile(
                                [P, QB], F32, tag="pt", padded_shape=[None, 512]
                            )
                            nc.tensor.matmul(
                                pt[:, c0:], k0t[:, kti * P:(kti + 1) * P],
                                qt[:, 0, c0:],
                                start=True, stop=False,
                            )
                            nc.tensor.matmul(
                                pt[:, c0:], k1t[:, kti * P:(kti + 1) * P],
                                qt[:, 1, c0:],
                                start=False, stop=True,
                            )
                            ep = ep_pool.tile([P, QB], BF16, tag="ep")
                            # exp(l/16 - 4): bf16 ep has f32's exponent range
                            # so the logit tails (observed ~17 scaled) cannot
                            # overflow; the -4 shift centers the weights and
                            # cancels exactly in the softmax normalization
                            nc.scalar.activation(
                                ep[:, c0:], pt[:, c0:],
                                mybir.ActivationFunctionType.Exp,
                                scale=1.0 / 16.0, bias=nbias[:],
                            )
                            if dt >= 0:
                                # only the first 128 columns of the trimmed
                                # range straddle the causal boundary
                                nc.vector.tensor_tensor(
                                    ep[:, c0:c0 + P], ep[:, c0:c0 + P],
                                    mask_t[:, dt, c0:c0 + P], MULT,
                                )
                            # denominator running sum (DVE, off the AV path)
                            if kti == 0:
                                nc.vector.tensor_copy(out=es[:], in_=ep[:])
                            else:
                                nc.vector.tensor_tensor(
                                    es[:, c0:], es[:, c0:], ep[:, c0:], ADD
                                )
                            if len(pend) == 2:
                                k0, e0, pc0 = pend.pop(0)
                                emit_av(k0, e0, pc0, k0 == 0, False)
                            pend.append((kti, ep, c0))
                        while pend:
                            k0, e0, pc0 = pend.pop(0)
                            emit_av(k0, e0, pc0, k0 == 0, k0 == nkt - 1)
                        pending.append((j, q0, av0, av1, es))
                flush_prev()

    nc.finalize()
    return nc


_DEINT = np.concatenate([np.arange(0, HD, 2), np.arange(1, HD, 2)])


def _prep_core_inputs(hidden_states, sinusoidal_pos, W_qkv):
    """Build the 8 per-core input dicts (b-major: core = b*4 + hg)."""
    sin = np.ascontiguousarray(sinusoidal_pos[:, :HD // 2])   # (S, 128)
    cos = np.ascontiguousarray(sinusoidal_pos[:, HD // 2:])
    sinT = np.ascontiguousarray(sin.T)                        # (128, S)
    cosT = np.ascontiguousarray(cos.T)

    masks = np.zeros((KTQ, P, QB), dtype=np.float32)
    k_rel = np.arange(P)[:, None]
    q_rel = np.arange(QB)[None, :]
    for ktl in range(KTQ):
        masks[ktl] = (k_rel + ktl * P <= q_rel).astype(np.float32)
    masks = masks.astype(ml_dtypes.bfloat16)

    hsT_b = [
        np.ascontiguousarray(hidden_states[b].T)
        .reshape(KK, P, S).astype(np.float16)
        for b in range(B)
    ]

    wt_hg = []
    for hg in range(H // NHC):  # 4 head groups
        heads = np.arange(NHC * hg, NHC * hg + NHC)
        feat = (heads[:, None] * HD + _DEINT[None, :]).reshape(-1)  # (1024,)
        cols = np.concatenate([3 * feat + 0, 3 * feat + 1, 3 * feat + 2])
        w = W_qkv[:, cols]                                    # (E, 3072)
        # -> (FT, P, KK, P): f-tile, partition(=contraction within chunk),
        #    chunk, feature-within-tile
        wt = np.ascontiguousarray(
            w.reshape(KK, P, FT, P).transpose(2, 1, 0, 3)
        ).astype(np.float16)
        wt_hg.append(wt)

    in_maps = []
    for c in range(8):
        b, hg = divmod(c, 4)
        in_maps.append({
            "hsT": hsT_b[b],
            "wt": wt_hg[hg],
            "sinT": sinT,
            "cosT": cosT,
            "masks": masks,
        })
    return in_maps


def _assemble(results):
    out = np.empty((B, S, E), dtype=np.float32)
    for c in range(8):
        b, hg = divmod(c, 4)
        heads = np.arange(NHC * hg, NHC * hg + NHC)
        feat = (heads[:, None] * HD + _DEINT[None, :]).reshape(-1)  # (1024,)
        core_out = results[c]["out"].reshape(2 * NHC * P, S)        # (1024, S)
        out[b][:, feat] = core_out.T
    return out


def _numpy_reference(hidden_states, sinusoidal_pos, attention_mask, W_qkv, b_qkv):
    """Exact fallback for off-spec inputs (nonzero bias / partial mask)."""
    b, s, _ = hidden_states.shape
    x = hidden_states.astype(np.float64)
    qkv = x @ W_qkv.astype(np.float64) + b_qkv.astype(np.float64)
    qkv = qkv.reshape(b, s, E, 3)
    q = qkv[..., 0].reshape(b, s, H, HD)
    k = qkv[..., 1].reshape(b, s, H, HD)
    v = qkv[..., 2].reshape(b, s, H, HD)
    sin, cos = np.split(sinusoidal_pos.astype(np.float64), 2, axis=-1)
    sin_pos = np.stack([sin, sin], axis=-1).reshape(s, HD)
    cos_pos = np.stack([cos, cos], axis=-1).reshape(s, HD)

    def rot(layer):
        rh = np.stack([-layer[..., 1::2], layer[..., ::2]], axis=-1)
        rh = rh.reshape(layer.shape)
        return layer * cos_pos[None, :, None, :] + rh * sin_pos[None, :, None, :]

    q, k, v = rot(q), rot(k), rot(v)
    causal = np.tril(np.ones((s, s), dtype=bool))[None, None]
    mask = np.logical_and(causal, attention_mask)
    logits = np.einsum("bqhd,bkhd->bhqk", q, k) / np.sqrt(HD)
    logits = np.where(mask, logits, -np.inf)
    logits -= logits.max(axis=-1, keepdims=True)
    w = np.exp(logits)
    w /= w.sum(axis=-1, keepdims=True)
    o = np.einsum("bhqk,bkhd->bqhd", w, v)
    return o.reshape(b, s, E).astype(np.float32)


_NC_CACHE = []


def kernel(hidden_states, sinusoidal_pos, attention_mask, W_qkv, b_qkv):
    hidden_states = np.asarray(hidden_states, dtype=np.float32)
    sinusoidal_pos = np.asarray(sinusoidal_pos, dtype=np.float32)
    attention_mask = np.asarray(attention_mask)
    W_qkv = np.asarray(W_qkv, dtype=np.float32)
    b_qkv = np.asarray(b_qkv, dtype=np.float32)

    if not bool(attention_mask.astype(bool).all()) or bool(np.any(b_qkv)):
        return _numpy_reference(
            hidden_states, sinusoidal_pos, attention_mask, W_qkv, b_qkv
        )

    if not _NC_CACHE:
        _NC_CACHE.append(build_nc())
    nc = _NC_CACHE[0]
    in_maps = _prep_core_inputs(hidden_states, sinusoidal_pos, W_qkv)
    res = run_bass_kernel_spmd(nc, in_maps, core_ids=list(range(8)))
    return _assemble(res.results)
